# revision 2
# baseline (speedup 1.0000x reference)
"""Trainium2 Bass kernel for sliding-window (±64) multi-head attention.

Reference computation (seq=4096, hidden=768, 12 heads x 64, RoPE, window 128):
    qkv = qkv_weight @ x ; q,k = rope(q,k) ; scores = q^T k / 8 + band_mask
    attn = softmax(scores) @ v ; out = out_weight @ attn

Sharding: sequence-parallel over 8 cores. Core c owns queries
[512c, 512c+512) and computes K/V over the haloed span [512c-64, 512c+576)
(zero-padded at the sequence edges; padding is killed by the band mask).
Each core runs an identical Bass program on different data; the full output
is reassembled on host by concatenation (no collectives needed).

Key structural choices (all bf16 matmuls; fp8 fails the 2e-2 rel-err gate):
- One consolidated DMA per input tensor (10 total) instead of 36.
- RoPE without evacuating the projection PSUM: since cos/sin are 32-periodic
  across each head's rotation pairs, rot(q*sin) == rot(q)*sin, so
  q_rope = q*cos + PERMS^T @ (q*sin); the two elementwise multiplies read
  the projection PSUM directly (DVE/Pool) and the rotation is a PE matmul.
- Softmax row chain per (head-pair, 128-query block): both heads' scores
  land in ONE PSUM bank -> one exp (ACT) -> fused mask-mult+row-sum
  (tensor_tensor_reduce, DVE) -> reciprocal + 4x-mode tensor_scalar
  normalize (DVE).
- P^T for the PV matmul comes from the DMA xbar transpose engine
  (dma_start_transpose), not PE: per unit one DMA does all four 128x128
  block transposes into a stride-padded SBUF tile.
- Band mask is 3 shared [128,256] slots (first block / interior / last
  block); the output DMA is bf16 and upcast on host.
"""

import os
import sys

import numpy as np

for _p in ("/opt/trn_rl_repo",):
    if _p not in sys.path and os.path.isdir(_p):
        sys.path.insert(0, _p)

import ml_dtypes

import concourse.bass as bass
import concourse.bacc as bacc
import concourse.tile as tile
from concourse import mybir
from concourse.bass_utils import run_bass_kernel_spmd

F32 = mybir.dt.float32
BF16 = mybir.dt.bfloat16

N_CORES = 8
SEQ = 4096
S_CORE = SEQ // N_CORES  # 512 queries per core
HALO = 64                # window // 2
SPAN = S_CORE + 2 * HALO  # 640 keys per core
HID = 768
NH = 12
DH = 64
NCH = HID // 128         # 6 contraction chunks
NHP = NH // 2            # 6 head pairs
NQB = S_CORE // 128      # 4 query blocks per core
NSC = SPAN // 128        # 5 key chunks per core
KSPAN = 256              # key span per query block
PTS_PITCH = 136          # padded block pitch of the transposed-P tile

_BUILD_CACHE = {}


def _build(add_mask: bool, reps: int = 1):
    """Build + compile the per-core Bass program (shared by all 8 cores)."""
    nc = bacc.Bacc("TRN2", target_bir_lowering=False, debug=False, num_devices=N_CORES)

    xin_d = nc.dram_tensor("xin", [128, NCH * SPAN], BF16, kind="ExternalInput")
    wqt_d = nc.dram_tensor("wqt", [128, NHP * NCH * 128], BF16, kind="ExternalInput")
    wkt_d = nc.dram_tensor("wkt", [128, NHP * NCH * 128], BF16, kind="ExternalInput")
    wvt_d = nc.dram_tensor("wvt", [128, NCH * HID], BF16, kind="ExternalInput")
    wot_d = nc.dram_tensor("wot", [128, NCH * HID], BF16, kind="ExternalInput")
    cos_d = nc.dram_tensor("cosb", [128, SPAN], BF16, kind="ExternalInput")
    sin_d = nc.dram_tensor("sinb", [128, SPAN], BF16, kind="ExternalInput")
    perm_d = nc.dram_tensor("perms", [128, 128], BF16, kind="ExternalInput")
    mask_d = nc.dram_tensor("maskb", [128, 3 * KSPAN], BF16, kind="ExternalInput")
    if add_mask:
        maskf_d = nc.dram_tensor(
            "maskf", [128, NQB * 2 * KSPAN], F32, kind="ExternalInput"
        )
    out_d = nc.dram_tensor("out", [128, NCH * S_CORE], BF16, kind="ExternalOutput")

    mult = mybir.AluOpType.mult
    addop = mybir.AluOpType.add
    exp = mybir.ActivationFunctionType.Exp

    with tile.TileContext(nc) as tc:
        from contextlib import ExitStack

        for _rep in range(reps):
          with ExitStack() as ctx:
            const = ctx.enter_context(tc.tile_pool(name="const", bufs=1))
            sb = ctx.enter_context(tc.tile_pool(name="sb", bufs=1))
            rope_p = ctx.enter_context(tc.tile_pool(name="rope", bufs=4))
            attnp = ctx.enter_context(tc.tile_pool(name="attnp", bufs=6))
            scal = ctx.enter_context(tc.tile_pool(name="scal", bufs=6))
            outp = ctx.enter_context(tc.tile_pool(name="outp", bufs=2))
            ps_proj = ctx.enter_context(
                tc.tile_pool(name="ps_proj", bufs=2, space="PSUM")
            )
            ps_att = ctx.enter_context(
                tc.tile_pool(name="ps_att", bufs=4, space="PSUM")
            )
            ps_o = ctx.enter_context(tc.tile_pool(name="ps_o", bufs=2, space="PSUM"))
            ps_rot = ps_proj

            # ---- input DMAs, split along first-use boundaries ----
            # x per contraction chunk (projection MM k only needs chunk k);
            # q/k weights per head pair; wv per output half.
            XIN = const.tile([128, NCH * SPAN], BF16, tag="XIN")
            WQT = const.tile([128, NHP * NCH * 128], BF16, tag="WQT")
            WKT = const.tile([128, NHP * NCH * 128], BF16, tag="WKT")
            WVT = const.tile([128, NCH * HID], BF16, tag="WVT")
            COS = const.tile([128, SPAN], BF16, tag="COS")
            SIN = const.tile([128, SPAN], BF16, tag="SIN")
            PERMS = const.tile([128, 128], BF16, tag="PERMS")
            MB = const.tile([128, 3 * KSPAN], BF16, tag="MB")

            def dma_xin(k):
                nc.sync.dma_start(
                    out=XIN[:, k * SPAN : (k + 1) * SPAN],
                    in_=xin_d.ap()[:, k * SPAN : (k + 1) * SPAN],
                )

            def dma_whp(dst, src_d, hp):
                w = NCH * 128
                nc.sync.dma_start(
                    out=dst[:, hp * w : (hp + 1) * w],
                    in_=src_d.ap()[:, hp * w : (hp + 1) * w],
                )

            dma_whp(WQT, wqt_d, 0)
            for k in range(NCH):
                dma_xin(k)
            nc.sync.dma_start(out=COS[:], in_=cos_d.ap())
            nc.sync.dma_start(out=SIN[:], in_=sin_d.ap())
            nc.sync.dma_start(out=PERMS[:], in_=perm_d.ap())
            dma_whp(WKT, wkt_d, 0)
            # wv halves: strided (per-chunk column halves)
            for hf in range(2):
                w = HID // 2
                nc.sync.dma_start(
                    out=WVT[:].rearrange("p (c o) -> p c o", c=NCH)[
                        :, :, hf * w : (hf + 1) * w
                    ],
                    in_=wvt_d.ap().rearrange("p (c o) -> p c o", c=NCH)[
                        :, :, hf * w : (hf + 1) * w
                    ],
                )
            nc.sync.dma_start(out=MB[:], in_=mask_d.ap())
            if add_mask:
                MF = const.tile([128, NQB * 2 * KSPAN], F32, tag="MF")
                nc.sync.dma_start(out=MF[:], in_=maskf_d.ap())
            for hp_ in range(1, NHP):
                dma_whp(WQT, wqt_d, hp_)
                dma_whp(WKT, wkt_d, hp_)
            WOT = sb.tile([128, NCH * HID], BF16, tag="WOT")
            nc.sync.dma_start(out=WOT[:], in_=wot_d.ap())

            # persistent intermediates
            Qs = sb.tile([128, NHP * S_CORE], BF16, tag="Qs")   # [2hd, (hp, s)]
            Ks = sb.tile([128, NHP * SPAN], BF16, tag="Ks")     # [2hd, (hp, s)]
            VT = sb.tile([128, NSC * HID], BF16, tag="VT")      # [s, (chunk, hd)]
            AT = sb.tile([128, NCH * S_CORE], BF16, tag="AT")   # [c, (cchunk, s)]

            def xs(k, lo, w):
                return XIN[:, k * SPAN + lo : k * SPAN + lo + w]

            # ---- V^T projection: VT[s, hd] per 128-key chunk (bf16) ----
            def vt_unit(sc):
                for hf in range(2):
                    w = HID // 2  # 384
                    vp = ps_proj.tile([128, w], F32, tag="proj")
                    for k in range(NCH):
                        nc.tensor.matmul(
                            vp[:],
                            xs(k, sc * 128, 128),
                            WVT[:, k * HID + hf * w : k * HID + (hf + 1) * w],
                            start=(k == 0),
                            stop=(k == NCH - 1),
                        )
                    nc.scalar.copy(
                        VT[:, sc * HID + hf * w : sc * HID + (hf + 1) * w], vp[:]
                    )

            # ---- per head pair: project Q,K then rope, in three phases so
            # the in-order PE queue never waits on elementwise results:
            #   phase 1 (proj_mm): q/k projection matmuls -> PSUM
            #   phase 2 (rope_mults): m1 = p*cos, m2 = p*sin (reads PSUM)
            #   phase 3 (rope_rot): rot = PERMS^T @ m2 (PE), dst = m1 + rot
            # PERMS^T commutes with the sin multiply because cos/sin are
            # 32-periodic in d.
            rope_st = {}

            def proj_mm(hp):
                wq = WQT[:, hp * NCH * 128 : (hp + 1) * NCH * 128]
                wk = WKT[:, hp * NCH * 128 : (hp + 1) * NCH * 128]
                blocks = []
                qp = ps_proj.tile([128, S_CORE], F32, tag="proj")
                for k in range(NCH):
                    nc.tensor.matmul(
                        qp[:],
                        wq[:, k * 128 : (k + 1) * 128],
                        xs(k, HALO, S_CORE),
                        start=(k == 0),
                        stop=(k == NCH - 1),
                    )
                blocks.append((qp, HALO, S_CORE,
                               Qs[:, hp * S_CORE : (hp + 1) * S_CORE]))
                for half in range(2):
                    w = SPAN // 2  # 320
                    kp = ps_proj.tile([128, w], F32, tag="proj")
                    for k in range(NCH):
                        nc.tensor.matmul(
                            kp[:],
                            wk[:, k * 128 : (k + 1) * 128],
                            xs(k, half * w, w),
                            start=(k == 0),
                            stop=(k == NCH - 1),
                        )
                    blocks.append(
                        (kp, half * w, w,
                         Ks[:, hp * SPAN + half * w : hp * SPAN + (half + 1) * w])
                    )
                rope_st[hp] = blocks

            def rope_mults(hp):
                # evacuate the projection PSUM once (ACT), then the cos/sin
                # multiplies run bf16 SBUF-only on Pool/DVE (GPSIMD cannot
                # touch PSUM)
                out = []
                for i, (p, lo, w, dst) in enumerate(rope_st[hp]):
                    qsb = rope_p.tile([128, S_CORE], BF16, tag="qsb")
                    nc.scalar.copy(qsb[:, :w], p[:])
                    m1 = rope_p.tile([128, S_CORE], BF16, tag="m1")
                    m2 = rope_p.tile([128, S_CORE], BF16, tag="m2")
                    nc.gpsimd.tensor_tensor(
                        m1[:, :w], qsb[:, :w], COS[:, lo : lo + w], op=mult
                    )
                    nc.vector.tensor_tensor(
                        m2[:, :w], qsb[:, :w], SIN[:, lo : lo + w], op=mult
                    )
                    out.append((m1, m2, w, dst))
                rope_st[hp] = out

            def rope_rot(hp):
                for i, (m1, m2, w, dst) in enumerate(rope_st[hp]):
                    rot = ps_proj.tile([128, S_CORE], F32, tag="proj")
                    nc.tensor.matmul(
                        rot[:, :w], PERMS[:], m2[:, :w], start=True, stop=True
                    )
                    nc.vector.tensor_tensor(dst, m1[:, :w], rot[:, :w], op=addop)
                del rope_st[hp]

            def proj_hp(hp):
                proj_mm(hp)
                rope_mults(hp)
                rope_rot(hp)

            # ---- attention: 5-stage modulo software pipeline over the 24
            # (head-pair, query-block) units ----
            def stage_scores(st):
                hp, qb = st["hp"], st["qb"]
                ss = []
                for h in range(2):
                    # full-bank tile (512 f32) so two tiles never share a
                    # PSUM bank; only the first KSPAN columns are used
                    s1 = ps_att.tile([128, 2 * KSPAN], F32, tag="att",
                                     name=f"s_{hp}_{qb}_{h}")
                    nc.tensor.matmul(
                        s1[:, :KSPAN],
                        Qs[64 * h : 64 * (h + 1),
                           hp * S_CORE + qb * 128 : hp * S_CORE + (qb + 1) * 128],
                        Ks[64 * h : 64 * (h + 1),
                           hp * SPAN + qb * 128 : hp * SPAN + qb * 128 + KSPAN],
                        start=True,
                        stop=True,
                    )
                    ss.append(s1)
                st["sc"] = ss

            def stage_exp(st):
                praw = attnp.tile([128, 2 * KSPAN], BF16, tag="praw")
                moff = st["qb"] * 2 * KSPAN
                for h in range(2):
                    sh = st["sc"][h][:, :KSPAN]
                    if add_mask:
                        nc.vector.tensor_tensor(
                            sh, sh,
                            MF[:, moff + h * KSPAN : moff + (h + 1) * KSPAN],
                            op=addop,
                        )
                    nc.scalar.activation(
                        praw[:, h * KSPAN : (h + 1) * KSPAN], sh, exp
                    )
                st["praw"] = praw
                del st["sc"]

            def stage_dve(st):
                qb = st["qb"]
                slot = 0 if qb == 0 else (2 if qb == NQB - 1 else 1)
                praw = st["praw"]
                P = attnp.tile([128, 2 * KSPAN], BF16, tag="P")
                ssum = scal.tile([128, 2], F32, tag="ssum")
                for h in range(2):
                    nc.vector.scalar_tensor_tensor(
                        out=P[:, h * KSPAN : (h + 1) * KSPAN],
                        in0=praw[:, h * KSPAN : (h + 1) * KSPAN],
                        scalar=1.0,
                        in1=MB[:, slot * KSPAN : (slot + 1) * KSPAN],
                        op0=mult,
                        op1=mult,
                        accum_out=ssum[:, h : h + 1],
                    )
                rr = scal.tile([128, 2], F32, tag="rr")
                nc.vector.reciprocal(rr[:], ssum[:])
                P2n = attnp.tile([128, 2 * KSPAN], BF16, tag="Pn")
                for h in range(2):
                    eng = nc.vector
                    eng.tensor_scalar_mul(
                        P2n[:, h * KSPAN : (h + 1) * KSPAN],
                        P[:, h * KSPAN : (h + 1) * KSPAN],
                        rr[:, h : h + 1],
                    )
                st["P2n"] = P2n
                del st["praw"]

            def stage_pt(st):
                # all four 128x128 block transposes in one xbar DMA. The out
                # AP is declared with a padded 136 pitch (keeps it 3D after
                # opt), but the HW xbar writes the transposed blocks PACKED
                # at pitch 128: pts[j, blk*128 + q] = P2n[q, blk*128 + j].
                P2n = st["P2n"]
                ptsp = attnp.tile([128, 4 * PTS_PITCH], BF16, tag="pts")
                pts3 = ptsp[:].rearrange("p (b j) -> p b j", b=4)
                nc.sync.dma_start_transpose(out=pts3[:, :, 0:128], in_=P2n[:])
                st["pts"] = ptsp
                del st["P2n"]

            def stage_pv(st):
                hp, qb = st["hp"], st["qb"]
                if qb == 0:
                    o2s[hp] = ps_o.tile([128, S_CORE], F32, tag="o",
                                        name=f"o2_{hp}")
                o2 = o2s[hp]
                pts = st["pts"]
                for h in range(2):
                    hg = hp * 2 + h
                    osl = o2[64 * h : 64 * (h + 1), qb * 128 : (qb + 1) * 128]
                    tp = (0, 64 * h)
                    nc.tensor.matmul(
                        osl,
                        VT[:, qb * HID + hg * 64 : qb * HID + hg * 64 + 64],
                        pts[:, (2 * h) * 128 : (2 * h + 1) * 128],
                        start=True, stop=False, tile_position=tp,
                    )
                    nc.tensor.matmul(
                        osl,
                        VT[:, (qb + 1) * HID + hg * 64 : (qb + 1) * HID + hg * 64 + 64],
                        pts[:, (2 * h + 1) * 128 : (2 * h + 2) * 128],
                        start=False, stop=True, tile_position=tp,
                    )
                del st["pts"]
                if qb == NQB - 1:
                    nc.vector.tensor_copy(
                        AT[:, hp * S_CORE : (hp + 1) * S_CORE], o2[:]
                    )
                    del o2s[hp]

            o2s = {}

            PO1 = sb.tile([128, NCH * S_CORE], F32, tag="PO1")

            def outproj_part1():
                for oc in range(NCH):
                    ops = ps_proj.tile([128, S_CORE], F32, tag="proj")
                    for k in range(5):
                        nc.tensor.matmul(
                            ops[:],
                            WOT[:, k * HID + oc * 128 : k * HID + (oc + 1) * 128],
                            AT[:, k * S_CORE : (k + 1) * S_CORE],
                            start=(k == 0),
                            stop=(k == 4),
                        )
                    nc.scalar.copy(PO1[:, oc * S_CORE : (oc + 1) * S_CORE], ops[:])

            proj_mm(0)
            rope_mults(0)
            vt_unit(0)
            rope_rot(0)
            vt_unit(1)

            def stage_bubble(st):
                # spacing stage: gives the pt transpose DMA time to land
                # before pv consumes it
                pass

            units = [
                {"hp": hp, "qb": qb} for hp in range(NHP) for qb in range(NQB)
            ]
            stages = [stage_scores, stage_exp, stage_dve, stage_pt,
                      stage_bubble, stage_pv]
            NU = len(units)
            ND = len(stages)
            # extra work interleaved into the pipeline: project head pair
            # hp at steps 4(hp-1)+{0,1,2} (2 head pairs of lead), remaining
            # V^T chunks early (pv of (hp, qb) needs VT chunks qb, qb+1)
            extra = {
                0: [lambda: proj_mm(1)],
                1: [lambda: rope_mults(1), lambda: vt_unit(2)],
                2: [lambda: rope_rot(1), lambda: vt_unit(3)],
                3: [lambda: vt_unit(4)],
            }
            for hp_ in range(2, NHP):
                base = 4 * (hp_ - 1)
                extra.setdefault(base + 0, []).append(
                    lambda h=hp_: proj_mm(h))
                extra.setdefault(base + 1, []).append(
                    lambda h=hp_: rope_mults(h))
                extra.setdefault(base + 2, []).append(
                    lambda h=hp_: rope_rot(h))

            for step in range(NU + ND - 1):
                for k in range(ND):
                    idx = step - k
                    if 0 <= idx < NU:
                        stages[k](units[idx])
                for fn in extra.get(step, ()):
                    fn()
                if step == 5 * 4 - 1 + ND - 1:
                    outproj_part1()

            # ---- output projection tail: last contraction chunk + combine ----
            for oc in range(NCH):
                ops = ps_proj.tile([128, S_CORE], F32, tag="proj")
                for k in range(5, NCH):
                    nc.tensor.matmul(
                        ops[:],
                        WOT[:, k * HID + oc * 128 : k * HID + (oc + 1) * 128],
                        AT[:, k * S_CORE : (k + 1) * S_CORE],
                        start=(k == 5),
                        stop=(k == NCH - 1),
                    )
                ot = outp.tile([128, S_CORE], BF16, tag="ot")
                eng = nc.vector
                eng.scalar_tensor_tensor(
                    out=ot[:], in0=ops[:], scalar=1.0,
                    in1=PO1[:, oc * S_CORE : (oc + 1) * S_CORE],
                    op0=mult, op1=addop,
                )
                deng = nc.scalar if oc % 2 == 0 else nc.sync
                deng.dma_start(
                    out=out_d.ap()[:, oc * S_CORE : (oc + 1) * S_CORE], in_=ot[:]
                )

    nc.compile()
    return nc


def get_program(add_mask: bool, reps: int = 1):
    key = (add_mask, reps)
    if key not in _BUILD_CACHE:
        _BUILD_CACHE[key] = _build(add_mask, reps)
    return _BUILD_CACHE[key]


def _pack_chunked(a, nch, w):
    """[nch*128, w] row-major -> [128, nch*w] with chunk-major free dim."""
    return np.ascontiguousarray(
        a.reshape(nch, 128, w).transpose(1, 0, 2).reshape(128, nch * w)
    )


def _band_tile(qg, kg):
    """[128, 256] bf16 band mask tile for global query rows qg, key cols kg."""
    kvalid = (kg >= 0) & (kg < SEQ)
    band = (np.abs(kg[None, :] - qg[:, None]) <= HALO) & kvalid[None, :]
    return band.astype(np.float32)


def prep_core_inputs(core, xs, pos, am, qkv_weight, out_weight, add_mask):
    """Build the per-core input map (numpy) for one core."""
    start = S_CORE * core - HALO
    idx = np.arange(start, start + SPAN)
    valid = (idx >= 0) & (idx < SEQ)

    Xs = np.zeros((HID, SPAN), np.float32)
    Xs[:, valid] = xs[:, idx[valid]]

    pspan = np.zeros((SPAN,), np.float32)
    pspan[valid] = pos[idx[valid]]
    invf = (
        1.0 / (10000.0 ** (np.arange(0, DH, 2, dtype=np.float32) / np.float32(DH)))
    ).astype(np.float32)
    f = pspan[None, :] * invf[:, None]  # [32, SPAN]
    COSb = np.tile(np.cos(f), (4, 1)).astype(ml_dtypes.bfloat16)
    SINb = np.tile(np.sin(f), (4, 1)).astype(ml_dtypes.bfloat16)

    # signed rotate-half permutation: (PERMS.T @ q)[d] = rot_half(q)[d]
    di = np.arange(128)
    lo = (di % 64) < 32
    src = np.where(lo, di + 32, di - 32)
    sgn = np.where(lo, -1.0, 1.0).astype(np.float32)
    PERMS = np.zeros((128, 128), np.float32)
    PERMS[src, di] = sgn

    # 3 mask slots: qb0 variant, interior, qb3 variant
    mb = np.zeros((128, 3, KSPAN), np.float32)
    for slot, qb in ((0, 0), (1, 1), (2, NQB - 1)):
        qg = S_CORE * core + 128 * qb + np.arange(128)
        kg = S_CORE * core + 128 * qb - HALO + np.arange(KSPAN)
        mb[:, slot] = _band_tile(qg, kg)

    mf = None
    if add_mask:
        mf = np.full((128, NQB, 2, KSPAN), -10000.0, np.float32)
        for qb in range(NQB):
            qg = S_CORE * core + 128 * qb + np.arange(128)
            kg = S_CORE * core + 128 * qb - HALO + np.arange(KSPAN)
            kvalid = (kg >= 0) & (kg < SEQ)
            band = (np.abs(kg[None, :] - qg[:, None]) <= HALO) & kvalid[None, :]
            amband = np.zeros((128, KSPAN), np.float32)
            amband[:, kvalid] = am[np.ix_(qg, kg[kvalid])]
            m = np.where(band, amband, -10000.0)
            mf[:, qb, 0, :] = m
            mf[:, qb, 1, :] = m

    wq = qkv_weight[0:HID] * np.float32(DH**-0.5)
    wk = qkv_weight[HID : 2 * HID]
    wv = qkv_weight[2 * HID : 3 * HID]

    def packw(w):
        return _pack_chunked(
            np.ascontiguousarray(w.T.astype(ml_dtypes.bfloat16)), NCH, HID
        )

    def packw_hp(w):
        # [c, o] -> [128, (hp, cchunk, 128)] so per-head-pair lhsT slices are
        # contiguous in the free dimension
        wt = np.ascontiguousarray(w.T.astype(ml_dtypes.bfloat16))  # [768c, 768o]
        a = wt.reshape(NCH, 128, NHP, 128)  # (cchunk, p, hp, n)
        return np.ascontiguousarray(
            a.transpose(1, 2, 0, 3).reshape(128, NHP * NCH * 128)
        )

    in_map = {
        "xin": _pack_chunked(Xs.astype(ml_dtypes.bfloat16), NCH, SPAN),
        "wqt": packw_hp(wq),
        "wkt": packw_hp(wk),
        "wvt": packw(wv),
        "wot": packw(out_weight),
        "cosb": COSb,
        "sinb": SINb,
        "perms": PERMS.astype(ml_dtypes.bfloat16),
        "maskb": mb.reshape(128, 3 * KSPAN).astype(ml_dtypes.bfloat16),
    }
    if add_mask:
        in_map["maskf"] = np.ascontiguousarray(mf.reshape(128, NQB * 2 * KSPAN))
    return in_map


def prep_all_inputs(x, position_ids, attention_mask, qkv_weight, out_weight):
    xs = np.asarray(x, dtype=np.float32)[0, :, 0, :]  # [768, 4096]
    pos = np.asarray(position_ids)[0].astype(np.float32)
    am = np.asarray(attention_mask, dtype=np.float32)[0, 0]
    qkv_w = np.asarray(qkv_weight, dtype=np.float32)
    out_w = np.asarray(out_weight, dtype=np.float32)
    add_mask = bool(np.any(am))
    in_maps = [
        prep_core_inputs(c, xs, pos, am, qkv_w, out_w, add_mask)
        for c in range(N_CORES)
    ]
    return in_maps, add_mask


def assemble_output(results):
    cols = []
    for c in range(N_CORES):
        o = np.asarray(results[c]["out"]).astype(np.float32)  # [128, 6*512] bf16
        cols.append(o.reshape(128, NCH, S_CORE).transpose(1, 0, 2).reshape(HID, S_CORE))
    full = np.concatenate(cols, axis=1)  # [768, 4096]
    return np.ascontiguousarray(full.reshape(1, HID, 1, SEQ), dtype=np.float32)


def kernel(**inputs):
    in_maps, add_mask = prep_all_inputs(
        inputs["x"],
        inputs["position_ids"],
        inputs["attention_mask"],
        inputs["qkv_weight"],
        inputs["out_weight"],
    )
    nc = get_program(add_mask)
    res = run_bass_kernel_spmd(nc, in_maps, core_ids=list(range(N_CORES)))
    return assemble_output(res.results)
